# revision 60
# baseline (speedup 1.0000x reference)
"""Trainium2 Bass kernel: single-head attention with RoPE and the reference's
multiplicative causal mask (masked logits stay 0 -> exp(0)=1, so masked
positions contribute exp(0)=1 to softmax -- attention is dense over the
upper triangle too, but those probabilities are a constant 1/Z).

Sharding: 8 cores = 4 batches x 2 row-parity halves. Core (b, h) owns the
interleaved rows x[b, h::2] -- with this split the causal-mask tile classes
are identical on every core, so fully-masked S^T tiles are skipped
statically (same SPMD graph everywhere) and their P==1 contribution enters
as a per-dout constant (onesum) plus a denominator offset.

Per core: project K (dlow-outer, weight panels loaded once), AllGather
roped K within the 2-core pair (single collective on the sync queue,
hidden under the V projection), project V (wb-outer so Wv column blocks
stream in late), AllGather V (hidden under the Q projection), project Q
with cos/sin reused from SBUF, then S^T = K@Q^T, P = exp(mask*S^T/sqrt(S)),
O^T = V^T@P^T / denom. Output is bf16 O^T per core; the host upcasts,
transposes and reassembles.

S^T skipping runs at 128-row granularity: chunk jc (j-range m'=jc%8) only
computes own-row columns i >= 128*m' in one merged chain per jc (split into
<=512-wide PSUM pieces); for odd m' the leading 128 columns of quarter
m'//2 are memset to exp(0)=1 so the per-quarter PV chains and denominators
stay exact. PV runs one 16-MM chain per dout chunk into a [128,1024] PSUM
tile with column ranges narrowing by mask class.
"""

import sys

for _p in ("/opt/trn_rl_repo", "/root/.axon_site/_ro/trn_rl_repo"):
    if _p not in sys.path:
        sys.path.append(_p)

import math

import ml_dtypes
import numpy as np

BF16 = ml_dtypes.bfloat16

B, S, D = 4, 2048, 2048
NOWN = 1024  # query rows per core
P = 128  # partitions
KD = D // P  # 16 feature chunks
NCJ = S // P  # 16 key chunks
N_CORES = 8
PAIRS = [[0, 1], [2, 3], [4, 5], [6, 7]]
FB = 512  # matmul moving free-dim block
NB = NOWN // FB  # 2 blocks of own rows
SCALE = 1.0 / math.sqrt(S)  # reference scales by sqrt(seq_len), not sqrt(D)

QW = 256  # quarter width (denominator / PV-scale granularity)
NQ = NOWN // QW  # 4 quarters


def _mp_of(jc):
    return jc % 8  # 128-granular mask class: columns i >= 128*m' computed


def _m_of(jc):
    return (jc % 8) // 2  # quarter-granular class (PV / denominators)


# chunks contributing computed S tiles for quarter q
def _comp(q):
    return [jc for jc in range(NCJ) if _m_of(jc) <= q]


# onesum stages: chunks that become skipped when stepping down a quarter
OS_STAGES = [
    [jc for jc in range(NCJ) if _m_of(jc) == 3],  # skipped for q<=2
    [jc for jc in range(NCJ) if _m_of(jc) == 2],  # additionally for q<=1
    [jc for jc in range(NCJ) if _m_of(jc) == 1],  # additionally for q==0
]

_CACHE = {}


def _build():
    import concourse.bass as bass  # noqa: F401
    import concourse.tile as tile
    from concourse import bacc, mybir

    f32 = mybir.dt.float32
    bf16 = mybir.dt.bfloat16

    nc = bacc.Bacc(
        "TRN2", target_bir_lowering=False, debug=False, num_devices=N_CORES
    )

    x_ext = nc.dram_tensor("x_t", [P, KD, NOWN], bf16, kind="ExternalInput").ap()
    wq_ext = nc.dram_tensor("wq", [KD, P, KD, P], bf16, kind="ExternalInput").ap()
    wk_ext = nc.dram_tensor("wk", [KD, P, KD, P], bf16, kind="ExternalInput").ap()
    wv_ext = nc.dram_tensor("wv", [P, KD, D], bf16, kind="ExternalInput").ap()
    cos_ext = nc.dram_tensor("cos_t", [KD, P, NOWN], bf16, kind="ExternalInput").ap()
    sin_ext = nc.dram_tensor("sin_t", [KD, P, NOWN], bf16, kind="ExternalInput").ap()
    mask_ext = nc.dram_tensor("mask_t", [NCJ, P, P], bf16, kind="ExternalInput").ap()
    sel6_ext = nc.dram_tensor("sel6", [6, 3], bf16, kind="ExternalInput").ap()
    out_ext = nc.dram_tensor("out", [D, NOWN], bf16, kind="ExternalOutput").ap()

    with tile.TileContext(nc) as tc:
        # dnsum first (bottom of the PSUM stack) so the projection psum
        # pool above it can be released after Q while dnsum stays live
        dnsum = tc.alloc_tile_pool(name="dnsum", bufs=1, space="PSUM")
        psum = tc.alloc_tile_pool(name="psum", bufs=5, space="PSUM")
        with (
            tc.tile_pool(name="dram", bufs=1, space="DRAM") as dram,
            tc.tile_pool(name="persist", bufs=1) as persist,
            tc.tile_pool(name="tmp", bufs=5) as tmp,
            tc.tile_pool(name="strm", bufs=8) as strm,
        ):
            kt_local = dram.tile([NCJ // 2, P, KD, P], bf16)
            v_local = dram.tile([NCJ // 2, P, D], bf16)
            # per-stage V-colsum partials over own chunks, exchanged in a
            # third (tiny) AllGather so the onesum corrections never need
            # 1-row partition-reduce matmuls in the attention phase
            os_local = dram.tile([3 * D], bf16)
            os_g = dram.tile([2, 3 * D], bf16)
            # K gathered in four 1MB ops grouped by dout quarter: ktq[0]
            # (dc 0-3) and ktq[2] (dc 8-11) are complete once dlow 0-3
            # finish both row halves, so those two exchanges run during
            # late K instead of colliding with the V projection's
            # v_local writes
            ktq = [
                dram.tile([NCJ // 2, P, 4, P], bf16, name=f"ktq{i}")
                for i in range(4)
            ]
            ktq_g = [
                dram.tile([2, NCJ // 2, P, 4, P], bf16, name=f"ktq_g{i}")
                for i in range(4)
            ]
            v_ga = dram.tile([2, 4, P, D], bf16)
            v_gb = dram.tile([2, 4, P, D], bf16)

            def v_g(jc):
                h2, jcl = jc // 8, jc % 8
                return (v_ga if jcl < 4 else v_gb)[h2, jcl % 4]

            ones_col = persist.tile([P, 1], bf16)
            nc.vector.memset(ones_col, 1.0)
            ones_row = persist.tile([1, P], f32)
            nc.vector.memset(ones_row, 1.0)
            # stage-selector for the onesum split matmuls: row h2*3+si
            # selects stage column si
            sel6 = persist.tile([6, 3], bf16)
            nc.gpsimd.dma_start(out=sel6, in_=sel6_ext)

            # dummy full-array matmuls during the input-DMA head: the PE
            # activity monitor un-throttles after ~3.4us of sustained
            # work, so the first real chains run at 2.4GHz instead of 1.2
            warm = persist.tile([P, FB], bf16)
            nc.vector.memset(warm, 0.0)
            ps_w = psum.tile([P, FB], f32, tag="ps", name="warm")
            for _ in range(12):
                nc.tensor.matmul(
                    ps_w, lhsT=warm[:, 0:P], rhs=warm, start=True, stop=True
                )

            # round-robin engine cycling for small DMA triggers, so no
            # single engine queue saturates on dispatch cost
            _rr = [0]
            _rr_engines = [nc.gpsimd, nc.sync, nc.scalar]

            def rr_dma(out, in_):
                _rr_engines[_rr[0] % 3].dma_start(out=out, in_=in_)
                _rr[0] += 1

            # x in 16 half-tiles [P, 2, FB]
            x_pool = tc.alloc_tile_pool(name="x_pool", bufs=1)
            x_ts = [
                [
                    x_pool.tile([P, 2, FB], bf16, name=f"x_sb{kg}_{h}")
                    for h in range(NB)
                ]
                for kg in range(8)
            ]

            def x_nb(k, nbi):
                # [P, FB] block of x chunk k, columns nbi*FB..
                return x_ts[k // 2][nbi][:, k % 2, :]

            def x_col(k, ncc):
                # [P, P] chunk-column ncc of x chunk k
                h, r = ncc // 4, ncc % 4
                return x_ts[k // 2][h][:, k % 2, r * P : (r + 1) * P]

            # cos/sin loaded once (full row range) per dlow and kept in
            # SBUF for both the K and Q projections.  The reference RoPE
            # table is concat([freqs, freqs]): chunk dlow+8 is
            # bit-identical to chunk dlow, so only the low half is loaded.
            cs_pool = tc.alloc_tile_pool(name="cs_pool", bufs=1)
            cs_all = {}

            def cs_alloc(dlow):
                tiles = [
                    cs_pool.tile([P, NOWN], bf16, name=f"cs_ct{dlow}"),
                    cs_pool.tile([P, NOWN], bf16, name=f"cs_st{dlow}"),
                ]
                cs_all[dlow] = tiles
                return tiles

            # ---- startup DMA front, ordered just-in-time ----
            # Items are emitted round-robin across the three trigger
            # queues in global need-order, so no queue's FIFO holds an
            # early-needed transfer behind a later-needed bulk one.
            sq_dma = rr_dma

            # wv_pool_a below wk_pool so releasing wkp after the K loop
            # stays LIFO while wv blocks 0/1 survive into the V phase
            wv_pool_a = tc.alloc_tile_pool(name="wv_pool_a", bufs=1)
            wkp = tc.alloc_tile_pool(name="wk_pool", bufs=8)

            def panel_tiles(wpool, dlow):
                dhigh = dlow + KD // 2
                w_lo = wpool.tile([P, KD, P], bf16, tag="wp", name=f"wlo{dlow}")
                w_hi = wpool.tile([P, KD, P], bf16, tag="wp", name=f"whi{dlow}")
                return w_lo, w_hi, dhigh

            # dlow=0 panels + x + cos/sin interleaved by first-use time:
            # the unit-0 lo chain consumes (wlo0 chunk k, x chunk k) in
            # lockstep, the hi chain follows one chain later, cos/sin are
            # first read by the DVE ~3.4us into the unit
            w_lo0, w_hi0, _dh0 = panel_tiles(wkp, 0)
            cs0 = cs_alloc(0)

            def x_dma(k, h):
                sq_dma(
                    x_ts[k // 2][h][:, k % 2 : k % 2 + 1, :],
                    x_ext[:, k : k + 1, h * FB : (h + 1) * FB],
                )

            for k in range(0, KD, 2):
                sq_dma(w_lo0[:, k : k + 2, :], wk_ext[0][:, k : k + 2, :])
                x_dma(k, 0)
                x_dma(k + 1, 0)
            def load_panels(wpool, w_ext, dlow):
                w_lo, w_hi, dhigh = panel_tiles(wpool, dlow)
                sq_dma(w_lo[:, 0:8, :], w_ext[dlow][:, 0:8, :])
                sq_dma(w_lo[:, 8:, :], w_ext[dlow][:, 8:, :])
                sq_dma(w_hi[:, 0:8, :], w_ext[dhigh][:, 0:8, :])
                sq_dma(w_hi[:, 8:, :], w_ext[dhigh][:, 8:, :])
                return w_lo, w_hi

            def cs_load_half(dlow, hi):
                if dlow not in cs_all:
                    cs_alloc(dlow)
                t = cs_all[dlow]
                off = FB if hi else 0
                sq_dma(t[0][:, off : off + FB], cos_ext[dlow][:, off : off + FB])
                sq_dma(t[1][:, off : off + FB], sin_ext[dlow][:, off : off + FB])

            # the first three units are the nb=0 halves of dlow 0-2, so
            # the 2MB x half-1 demand is deferred past the early DMA
            # delivery ramp; panels/cos-sin for dlow 1-2 load in between
            for k in range(0, KD, 4):
                sq_dma(w_hi0[:, k : k + 4, :], wk_ext[8][:, k : k + 4, :])
                if k == 8:
                    cs_load_half(0, False)
            panels_by = {0: (w_lo0, w_hi0)}
            panels_by[1] = load_panels(wkp, wk_ext, 1)
            cs_load_half(1, False)
            panels_by[2] = load_panels(wkp, wk_ext, 2)
            cs_load_half(2, False)
            for k in range(KD):
                x_dma(k, 1)
                if k == 9:
                    cs_load_half(0, True)
                if k == 12:
                    cs_load_half(1, True)
            cs_load_half(2, True)

            def rope_pair(panels, dlow, nb, cs_tiles, out_ap, post):
                """One (dlow, nb) unit: two projection chains + rope."""
                dhigh = dlow + KD // 2
                sl = slice(nb * FB, (nb + 1) * FB)
                cos_t, sin_t = (t[:, sl] for t in cs_tiles)
                w_lo, w_hi = panels
                ps_lo = psum.tile([P, FB], f32, tag="ps", name=f"plo{dlow}{nb}")
                for k in range(KD):
                    nc.tensor.matmul(
                        ps_lo,
                        lhsT=w_lo[:, k, :],
                        rhs=x_nb(k, nb),
                        start=(k == 0),
                        stop=(k == KD - 1),
                    )
                ps_hi = psum.tile([P, FB], f32, tag="ps", name=f"phi{dlow}{nb}")
                for k in range(KD):
                    nc.tensor.matmul(
                        ps_hi,
                        lhsT=w_hi[:, k, :],
                        rhs=x_nb(k, nb),
                        start=(k == 0),
                        stop=(k == KD - 1),
                    )
                # rope low half: out = lo*cos_l - hi*sin_l
                t1 = tmp.tile([P, FB], f32, tag="t", name=f"t1{dlow}{nb}")
                nc.vector.tensor_mul(t1, ps_lo, cos_t)
                t2 = tmp.tile([P, FB], f32, tag="t", name=f"t2{dlow}{nb}")
                nc.vector.tensor_mul(t2, ps_hi, sin_t)
                o_lo = out_ap(dlow, nb)
                nc.vector.tensor_sub(o_lo, t1, t2)
                if post is not None:
                    post(dlow, nb, o_lo)
                # rope high half: out = hi*cos + lo*sin (the table's high
                # half equals its low half, so the same tiles serve both)
                t3 = tmp.tile([P, FB], f32, tag="t", name=f"t3{dlow}{nb}")
                nc.vector.tensor_mul(t3, ps_hi, cos_t)
                t4 = tmp.tile([P, FB], f32, tag="t", name=f"t4{dlow}{nb}")
                nc.vector.tensor_mul(t4, ps_lo, sin_t)
                o_hi = out_ap(dhigh, nb)
                nc.vector.tensor_add(o_hi, t3, t4)
                if post is not None:
                    post(dhigh, nb, o_hi)

            # ---- K projection + rope -> kt_local ----
            def k_out(dc, nb):
                return strm.tile([P, FB], bf16, tag="ro", name=f"kt_{dc}_{nb}")

            def k_post(dc, nb, t):
                # one strided DMA per [P, 512] tile: dst iterates
                # [p][jj][j] so the SBUF side keeps partitions first
                dst = ktq[dc // 4][
                    nb * 4 : (nb + 1) * 4, :, dc % 4, :
                ].transpose([1, 0, 2])
                rr_dma(dst, t.rearrange("p (jj j) -> p jj j", jj=4))

            # wv column blocks [P, KD, FB]; blocks 0/1 start loading during
            # the last K units, 2/3 at V start.  All wv loads ride the
            # scalar queue so gpsimd/sync stay clear for v_local writes.
            wv_blks = {}

            def emit_wv_load(pool, wb, eng):
                t = pool.tile([P, KD, FB], bf16, name=f"wv_sb{wb}")
                eng.dma_start(out=t, in_=wv_ext[:, :, wb * FB : (wb + 1) * FB])
                wv_blks[wb] = t

            # unit order (0,0),(1,0),(2,0),(0,1),(1,1),(2,1) then dlow
            # 3..7 paired; panels/cs prefetch with 2-unit lookahead
            units = [(0, 0), (1, 0), (2, 0), (0, 1), (1, 1), (2, 1)] + [
                (d, n) for d in range(3, KD // 2) for n in range(NB)
            ]
            cs_hi_done = {0, 1, 2}
            for i, (d, nb) in enumerate(units):
                if i + 2 < len(units):
                    d2, n2 = units[i + 2]
                    if n2 == 0 and d2 not in panels_by:
                        panels_by[d2] = load_panels(wkp, wk_ext, d2)
                        cs_load_half(d2, False)
                    if n2 == 1 and d2 not in cs_hi_done:
                        cs_hi_done.add(d2)
                        cs_load_half(d2, True)
                if (d, nb) == (6, 0):
                    emit_wv_load(wv_pool_a, 0, nc.scalar)
                if (d, nb) == (7, 0):
                    emit_wv_load(wv_pool_a, 1, nc.scalar)
                rope_pair(panels_by[d], d, nb, cs_all[d], k_out, k_post)
                if i == 7:
                    # dc 0-3 and 8-11 fully written: exchange them now,
                    # under the rest of the K projection
                    for q4 in (0, 2):
                        nc.gpsimd.collective_compute(
                            "AllGather",
                            mybir.AluOpType.bypass,
                            replica_groups=PAIRS,
                            ins=[ktq[q4].opt()],
                            outs=[ktq_g[q4].opt()],
                        )
            wkp.release()

            for q4 in (1, 3):
                nc.gpsimd.collective_compute(
                    "AllGather",
                    mybir.AluOpType.bypass,
                    replica_groups=PAIRS,
                    ins=[ktq[q4].opt()],
                    outs=[ktq_g[q4].opt()],
                )

            # ---- V projection (wb-outer; Wv blocks stream in) ----
            # all three trigger queues: ~600ns per trigger means two
            # queues can't even issue the 64 v_local writes fast enough
            # at the V tail (scalar only carries two wv-block triggers)
            _vw = [0]
            _vw_engines = [nc.gpsimd, nc.sync, nc.scalar]

            def vw_dma(out, in_):
                _vw_engines[_vw[0] % 3].dma_start(out=out, in_=in_)
                _vw[0] += 1

            wv_pool_b = tc.alloc_tile_pool(name="wv_pool_b", bufs=1)
            emit_wv_load(wv_pool_b, 2, nc.scalar)
            emit_wv_load(wv_pool_b, 3, nc.scalar)
            ps_cs = [None]
            for wb in range(D // FB):
                for ncc in range(NCJ // 2):
                    ps_v = psum.tile([P, FB], f32, tag="ps")
                    for k in range(KD):
                        nc.tensor.matmul(
                            ps_v,
                            lhsT=x_col(k, ncc),
                            rhs=wv_blks[wb][:, k, :],
                            start=(k == 0),
                            stop=(k == KD - 1),
                        )
                    # deep ring: v_local writes stall ~12us when they
                    # overlap the K-AllGather's DMA traffic, and the ring
                    # must cover that without blocking PSUM drains
                    v_t = strm.tile([P, FB], bf16, tag="vo", bufs=12)
                    nc.vector.tensor_copy(v_t, ps_v)
                    vw_dma(v_local[ncc][:, wb * FB : (wb + 1) * FB], v_t)
                    # per-stage colsum of the own V chunks (m(ncc)>=1):
                    # dense 512-row MMs with a fixed ones lhsT
                    if ncc >= 2:
                        si = 3 - ncc // 2
                        if ncc % 2 == 0:
                            ps_cs[0] = dnsum.tile(
                                [1, FB], f32, tag="dn", name=f"cs{si}{wb}"
                            )
                        nc.tensor.matmul(
                            ps_cs[0],
                            lhsT=ones_col,
                            rhs=v_t,
                            start=(ncc % 2 == 0),
                            stop=(ncc % 2 == 1),
                        )
                        if ncc % 2 == 1:
                            oss = strm.tile(
                                [1, FB], bf16, tag="oss", bufs=2
                            )
                            nc.vector.tensor_copy(oss, ps_cs[0])
                            off = (si * 4 + wb) * FB
                            nc.sync.dma_start(
                                out=os_local[off : off + FB].unsqueeze(0),
                                in_=oss,
                            )
            wv_pool_b.release()
            wv_pool_a.release()

            for half, out_t in ((0, v_ga), (1, v_gb)):
                nc.gpsimd.collective_compute(
                    "AllGather",
                    mybir.AluOpType.bypass,
                    replica_groups=PAIRS,
                    ins=[v_local[half * 4 : (half + 1) * 4].opt()],
                    outs=[out_t.opt()],
                )
            nc.gpsimd.collective_compute(
                "AllGather",
                mybir.AluOpType.bypass,
                replica_groups=PAIRS,
                ins=[os_local.opt()],
                outs=[os_g.opt()],
            )

            # ---- Q projection + rope (cos/sin reused from SBUF) ----
            # qt + slab pools live on the right SBUF stack, created before
            # Q so they sit OUTSIDE the zone freed by x/cs: the kt slab
            # loads can then prefetch during the Q projection instead of
            # picking up a runtime dependency on the x/cs release
            qt_pool = tc.alloc_tile_pool(name="qt_pool", bufs=1, side="right")
            qt_sb = qt_pool.tile([P, KD, NOWN], bf16)
            slab = tc.alloc_tile_pool(name="slab", bufs=6, side="right")

            def q_out(dc, nb):
                return qt_sb[:, dc, nb * FB : (nb + 1) * FB]

            # wq panels on scalar only: gpsimd holds the v2 loads that wait
            # on the V-gather, sync holds the kt-slab loads that wait on
            # the K-gather -- panels must not queue behind either
            def load_panels_q(wpool, dlow):
                w_lo, w_hi, dhigh = panel_tiles(wpool, dlow)
                nc.scalar.dma_start(out=w_lo, in_=wq_ext[dlow])
                nc.scalar.dma_start(out=w_hi, in_=wq_ext[dhigh])
                return w_lo, w_hi

            with tc.tile_pool(name="wq_pool", bufs=6) as wqp:
                q_panels_next = load_panels_q(wqp, 0)
                for dlow in range(KD // 2):
                    panels = q_panels_next
                    if dlow + 1 < KD // 2:
                        q_panels_next = load_panels_q(wqp, dlow + 1)
                    for nb in range(NB):
                        rope_pair(panels, dlow, nb, cs_all[dlow], q_out, None)
            cs_pool.release()
            x_pool.release()

            # projection PSUM ring released; attention re-stacks PSUM as
            # a 2-slot S^T piece pool + 2-slot [P,1024] PV pool
            psum.release()
            at_psum = tc.alloc_tile_pool(name="at_psum", bufs=2, space="PSUM")
            pvp = tc.alloc_tile_pool(name="pvp", bufs=4, space="PSUM")

            # ---- Attention ----
            with (
                tc.tile_pool(name="v2_pool", bufs=1, side="right") as v2p,
                tc.tile_pool(name="pt_pool", bufs=1, side="right") as ptp,
                tc.tile_pool(name="mskp", bufs=2, side="right") as mskp,
                tc.tile_pool(name="outp", bufs=4, side="right") as outp,
                tc.tile_pool(name="smallp", bufs=1, side="right") as smallp,
            ):
                v2_sb = v2p.tile([P, NCJ, D], bf16)
                # staggered across the S^T groups (two per group, lowest
                # mask class first) so the slab loads that gate the S^T
                # chains never queue behind an 8MB v2 burst; PV only needs
                # v2 after the last group
                v2_order = [
                    jc
                    for m in range(NQ)
                    for jc in range(NCJ)
                    if _m_of(jc) == m
                ]
                _v2i = [0]

                def emit_v2_loads(n):
                    while n > 0 and _v2i[0] < NCJ:
                        jc = v2_order[_v2i[0]]
                        nc.gpsimd.dma_start(out=v2_sb[:, jc, :], in_=v_g(jc))
                        _v2i[0] += 1
                        n -= 1

                emit_v2_loads(2)
                pt_sb = ptp.tile([P, NCJ, NOWN], bf16)

                def s_chunk(jc, first=False):
                    """Merged S^T chain(s) for chunk jc: columns
                    [128*m', 1024) in <=512-wide PSUM pieces; mixed
                    128-col diagonal block gets the mask; for odd m' the
                    128 cols below get exp(0)=1 via memset."""
                    mp = _mp_of(jc)
                    w0 = P * mp
                    kt_slab = slab.tile([P, KD, P], bf16, tag="slab")
                    for q4 in range(4):
                        nc.sync.dma_start(
                            out=kt_slab[:, q4 * 4 : (q4 + 1) * 4, :],
                            in_=ktq_g[q4][jc // 8, jc % 8],
                        )
                    msk = mskp.tile([P, P], bf16, tag="m")
                    nc.scalar.dma_start(out=msk, in_=mask_ext[jc])
                    if mp % 2 == 1:
                        nc.vector.memset(pt_sb[:, jc, w0 - P : w0], 1.0)
                    pieces = [(w0, min(w0 + FB, NOWN))]
                    if w0 + FB < NOWN:
                        pieces.append((w0 + FB, NOWN))
                    for pi, (a, b) in enumerate(pieces):
                        # the very first chunk borrows the idle dnsum
                        # banks: at_psum reuses the released projection
                        # ring's addresses, so its first slots carry a
                        # wait on the full Q drain (~3us) that this hides
                        if first:
                            ps_s = dnsum.tile(
                                [P, b - a], f32,
                                tag="dn" if pi == 0 else "os",
                                name=f"ps_s{jc}{pi}",
                            )
                        else:
                            ps_s = at_psum.tile(
                                [P, b - a], f32, tag="ps", name=f"ps_s{jc}{pi}"
                            )
                        for k in range(KD):
                            nc.tensor.matmul(
                                ps_s,
                                lhsT=kt_slab[:, k, :],
                                rhs=qt_sb[:, k, a:b],
                                start=(k == 0),
                                stop=(k == KD - 1),
                            )
                        if pi == 0:
                            # mixed diagonal block [w0, w0+128)
                            tm = tmp.tile(
                                [P, P], f32, tag="tm", name=f"tm{jc}", bufs=4
                            )
                            nc.vector.tensor_mul(tm, ps_s[:, 0:P], msk)
                            nc.scalar.activation(
                                out=pt_sb[:, jc, w0 : w0 + P],
                                in_=tm,
                                func=mybir.ActivationFunctionType.Exp,
                                scale=SCALE,
                            )
                            if b > w0 + P:
                                nc.scalar.activation(
                                    out=pt_sb[:, jc, w0 + P : b],
                                    in_=ps_s[:, P:],
                                    func=mybir.ActivationFunctionType.Exp,
                                    scale=SCALE,
                                )
                        else:
                            nc.scalar.activation(
                                out=pt_sb[:, jc, a:b],
                                in_=ps_s,
                                func=mybir.ActivationFunctionType.Exp,
                                scale=SCALE,
                            )

                # denominators + reciprocals; skipped chunks contribute
                # (12 - 4q)*128 exact ones
                recips = [None] * NQ
                rbs = {}

                def emit_denom(q):
                    jcs = _comp(q)
                    ps_d = dnsum.tile([1, QW], f32, tag="dn", name=f"psd{q}")
                    for idx, jc in enumerate(jcs):
                        nc.tensor.matmul(
                            ps_d,
                            lhsT=ones_col,
                            rhs=pt_sb[:, jc, q * QW : (q + 1) * QW],
                            start=(idx == 0),
                            stop=(idx == len(jcs) - 1),
                        )
                    recip = smallp.tile([1, QW], f32, tag="rc", name=f"rc{q}", bufs=2)
                    nones = (12 - 4 * q) * P
                    if nones:
                        dfix = smallp.tile([1, QW], f32, tag="dfix", name=f"df{q}")
                        nc.vector.tensor_scalar_add(dfix, ps_d, float(nones))
                        nc.vector.reciprocal(recip, dfix)
                    else:
                        nc.vector.reciprocal(recip, ps_d)
                    recips[q] = recip

                def emit_rb(q):
                    # reciprocal broadcast via fp32 outer product; emitted
                    # one group after its denom so the DVE recip is done
                    ps_rb = at_psum.tile([P, QW], f32, tag="ps", name=f"prb{q}")
                    nc.tensor.matmul(
                        ps_rb, lhsT=ones_row, rhs=recips[q], start=True, stop=True
                    )
                    # distinct tag per q: all four broadcasts stay live
                    # until their q's scale pass at the end of the kernel
                    rb = smallp.tile([P, QW], f32, tag=f"rbs{q}", name=f"rb{q}")
                    nc.vector.tensor_copy(rb, ps_rb)
                    rbs[q] = rb

                # onesum stage sums arrive via the colsum AllGather; 16
                # small fp32 split matmuls land them per-partition as
                # [P, 3] column triples of one PSUM tile
                os6 = smallp.tile([6, D], bf16, tag="os6")
                nc.sync.dma_start(
                    out=os6, in_=os_g.rearrange("h (s d) -> (h s) d", s=3)
                )
                ps_os3 = dnsum.tile([P, KD, 3], f32, tag="os")

                def emit_os_split():
                    for dc in range(KD):
                        nc.tensor.matmul(
                            ps_os3[:, dc, :],
                            lhsT=os6[:, dc * P : (dc + 1) * P],
                            rhs=sel6,
                            start=True,
                            stop=True,
                        )

                # cumulative onesum sums on DVE:
                # q2 uses p0, q1 p0+p1, q0 p0+p1+p2
                os_of_q = {}

                def emit_os_combine():
                    parts = []
                    for si in range(3):
                        p_sb = smallp.tile(
                            [P, KD], f32, tag=f"osp{si}", name=f"osp{si}"
                        )
                        nc.vector.tensor_copy(p_sb, ps_os3[:, :, si])
                        parts.append(p_sb)
                    os1 = smallp.tile([P, KD], f32, tag="os1c")
                    nc.vector.tensor_add(os1, parts[0], parts[1])
                    os0 = smallp.tile([P, KD], f32, tag="os0c")
                    nc.vector.tensor_add(os0, os1, parts[2])
                    os_of_q[2] = parts[0]
                    os_of_q[1] = os1
                    os_of_q[0] = os0

                # S^T in mask-class order: after class 2q+1, the quarter-q
                # denominator inputs are complete, so the denom/recip/rb
                # work interleaves into the dense S^T stream
                for gp in range(8):
                    for jc in (gp, gp + 8):
                        s_chunk(jc, first=(gp == 0 and jc == 0))
                        emit_v2_loads(2)
                    if gp % 2 == 1:
                        q = (gp - 1) // 2
                        emit_denom(q)
                        if q > 0:
                            emit_rb(q - 1)
                    if gp == 6:
                        emit_os_split()
                        emit_os_combine()
                emit_v2_loads(NCJ)  # any remainder

                def pv_chain(dc, half):
                    # one chain per (dout chunk, 512-col half); chunks
                    # whose mask class starts inside the half accumulate
                    # into a column suffix of the same PSUM tile, so the
                    # widest (lowest-m) MMs run first with start=True
                    base = half * FB
                    mms = []
                    for m in range(NQ):
                        lo = max(m * QW, base)
                        if lo >= base + FB:
                            continue
                        for jc in range(NCJ):
                            if _m_of(jc) == m:
                                mms.append((lo, jc))
                    ps_o = pvp.tile(
                        [P, FB], f32, tag="pv", name=f"pso{dc}{half}"
                    )
                    for idx, (lo, jc) in enumerate(mms):
                        nc.tensor.matmul(
                            ps_o[:, lo - base :],
                            lhsT=v2_sb[:, jc, dc * P : (dc + 1) * P],
                            rhs=pt_sb[:, jc, lo : base + FB],
                            start=(idx == 0),
                            stop=(idx == len(mms) - 1),
                        )
                    return ps_o

                # sync is free once the kt slabs are in, so all three
                # trigger queues share the output drain
                _out_engines = [nc.gpsimd, nc.scalar, nc.sync]

                def emit_scale(q, dc, ps_o, base):
                    o_st = outp.tile([P, QW], bf16, tag="o", name=f"ost{q}{dc}")
                    sl = slice(q * QW - base, (q + 1) * QW - base)
                    if q in os_of_q:
                        nc.vector.scalar_tensor_tensor(
                            out=o_st,
                            in0=ps_o[:, sl],
                            scalar=os_of_q[q][:, dc : dc + 1],
                            in1=rbs[q],
                            op0=mybir.AluOpType.add,
                            op1=mybir.AluOpType.mult,
                        )
                    else:
                        nc.vector.tensor_mul(o_st, ps_o[:, sl], rbs[q])
                    _out_engines[dc % 3].dma_start(
                        out=out_ext[
                            dc * P : (dc + 1) * P, q * QW : (q + 1) * QW
                        ],
                        in_=o_st,
                    )

                # all short half-0 chains first: their scales need only
                # rb0/rb1, so the q=3 reciprocal + broadcast (which only
                # complete after the last S^T group) hide under ~20us of
                # half-0 work instead of stalling the first PV scales
                ch_order = [(dc, 0) for dc in range(KD)] + [
                    (dc, 1) for dc in range(KD)
                ]
                chains = {}
                for key in ch_order[:3]:
                    chains[key] = pv_chain(*key)
                # rb(3) emitted behind the first chains: its matmul then
                # reaches the PE only after the slow single-partition
                # q=3 reciprocal has finished on DVE, instead of idling
                # the PE at the S^T->PV transition; half-1 scales that
                # read it run ~20us later
                emit_rb(NQ - 1)

                for i, (dc, half) in enumerate(ch_order):
                    ps_o = chains.pop((dc, half))
                    for q in (2 * half, 2 * half + 1):
                        emit_scale(q, dc, ps_o, half * FB)
                    if i + 3 < len(ch_order):
                        chains[ch_order[i + 3]] = pv_chain(*ch_order[i + 3])
            slab.release()
            qt_pool.release()
            pvp.release()
            at_psum.release()
            dnsum.release()

    nc.compile()
    return nc


def _prep_inputs(x, cos, sin, Wq, Wk, Wv):
    """Host-side sharding/layout prep. Returns in_maps for 8 cores."""
    x = np.asarray(x, dtype=np.float32)
    cos = np.asarray(cos, dtype=np.float32)
    sin = np.asarray(sin, dtype=np.float32)

    def w_panels(w):
        # W.T [din, dout] -> [dc, p_din, k_din, c_dout] with d = k*128+p
        wt = np.ascontiguousarray(np.asarray(w, dtype=np.float32).T).astype(BF16)
        return np.ascontiguousarray(
            wt.reshape(KD, P, KD, P).transpose(2, 1, 0, 3)
        )

    wq_p = w_panels(Wq)
    wk_p = w_panels(Wk)
    # Wv.T [din, dout] -> [p, k, dout]
    wv_p = np.ascontiguousarray(
        np.asarray(Wv, dtype=np.float32)
        .T.astype(BF16)
        .reshape(KD, P, D)
        .transpose(1, 0, 2)
    )

    # global row index of gathered slot s: pair rank h2 = s // NOWN owns the
    # rows with parity h2, so j_global(s) = 2*(s % NOWN) + h2
    slot = np.arange(S, dtype=np.int64)
    j_global = 2 * (slot % NOWN) + slot // NOWN

    in_maps = []
    for c in range(N_CORES):
        b, h = divmod(c, 2)
        rows = slice(h, None, 2)  # interleaved rows: h, h+2, h+4, ...
        xt = np.ascontiguousarray(
            x[b, rows, :].T.astype(BF16).reshape(KD, P, NOWN).transpose(1, 0, 2)
        )
        cos_t = np.ascontiguousarray(cos[rows].T.astype(BF16).reshape(KD, P, NOWN))
        sin_t = np.ascontiguousarray(sin[rows].T.astype(BF16).reshape(KD, P, NOWN))
        i_global = 2 * np.arange(NOWN, dtype=np.int64) + h
        # per jc, only the mixed 128-col diagonal block needs mask data
        mask_t = np.empty((NCJ, P, P), dtype=BF16)
        for jc in range(NCJ):
            mp = jc % 8
            jg = j_global[jc * P : (jc + 1) * P][:, None]
            ig = i_global[mp * P : (mp + 1) * P][None, :]
            mask_t[jc] = (jg <= ig).astype(BF16)
        sel6 = np.zeros((6, 3), dtype=BF16)
        for j in range(6):
            sel6[j, j % 3] = 1
        in_maps.append(
            {
                "x_t": xt,
                "wq": wq_p,
                "wk": wk_p,
                "wv": wv_p,
                "cos_t": cos_t,
                "sin_t": sin_t,
                "mask_t": mask_t,
                "sel6": sel6,
            }
        )
    return in_maps


def _run(in_maps, trace=False, tmpdir=None):
    from concourse.bass_utils import run_bass_kernel_spmd

    if "nc" not in _CACHE:
        _CACHE["nc"] = _build()
    nc = _CACHE["nc"]
    return run_bass_kernel_spmd(
        nc, in_maps, list(range(N_CORES)), trace=trace, tmpdir=tmpdir
    )


def kernel(x, cos, sin, Wq, Wk, Wv):
    in_maps = _prep_inputs(x, cos, sin, Wq, Wk, Wv)
    res = _run(in_maps, trace=False)
    out = np.empty((B, S, D), dtype=np.float32)
    for c in range(N_CORES):
        b, h = divmod(c, 2)
        out[b, h::2, :] = res.results[c]["out"].astype(np.float32).T
    return out
